# revision 8
# baseline (speedup 1.0000x reference)
"""AgentAttention on 8 TRN2 NeuronCores — hand-written Bass/Tile kernel (v2).

Data-parallel over batch: 2 images per core, all parameters replicated.
Host prep: bf16 packing, pre-transposed x (xT), pooled+scaled agent-token
input (xpT), exp(position-bias) tables.
"""
import sys
sys.path.insert(0, '/opt/trn_rl_repo')

import numpy as np
import ml_dtypes

NPBF = ml_dtypes.bfloat16
B, C, H_, W_ = 16, 512, 56, 56
N_ = H_ * W_
HEADS, AGENT, POOL = 8, 49, 7
NCORES = 8


def _bilinear_resize_np(img, out_h, out_w):
    *lead, in_h, in_w = img.shape
    ys = (np.arange(out_h) + 0.5) * (in_h / out_h) - 0.5
    xs = (np.arange(out_w) + 0.5) * (in_w / out_w) - 0.5
    y0 = np.floor(ys).astype(np.int64)
    x0 = np.floor(xs).astype(np.int64)
    wy = (ys - y0).astype(np.float32)
    wx = (xs - x0).astype(np.float32)
    y0c, y1c = np.clip(y0, 0, in_h - 1), np.clip(y0 + 1, 0, in_h - 1)
    x0c, x1c = np.clip(x0, 0, in_w - 1), np.clip(x0 + 1, 0, in_w - 1)
    flat = img.reshape(-1, in_h, in_w)
    rows = flat[:, y0c, :] * (1 - wy)[None, :, None] + flat[:, y1c, :] * wy[None, :, None]
    out = rows[:, :, x0c] * (1 - wx)[None, None, :] + rows[:, :, x1c] * wx[None, None, :]
    return out.reshape(*lead, out_h, out_w).astype(np.float32)


def _padind():
    o = np.zeros((128, 4), np.float32)
    for j in range(2):
        o[64 * j + AGENT:64 * (j + 1), :] = 1.0
    return o


def _pack_w(w):
    # w: [Cout, Cin] fp32 -> lhsT tiles [128, 4, Cout] bf16 (wT[p,kt,m] = w[m,128kt+p])
    return np.ascontiguousarray(
        w.T.reshape(4, 128, w.shape[0]).transpose(1, 0, 2)).astype(NPBF)


def _prepare(inputs):
    x = np.asarray(inputs['x'], np.float32).reshape(B, N_, C)
    # xT: [B, 128, 4, N]  xT[b,p,kt,n] = x[b,n,128kt+p]
    xT = np.ascontiguousarray(
        x.reshape(B, N_, 4, 128).transpose(0, 3, 2, 1)).astype(NPBF)
    # pooled agent-token input, pre-scaled by hd^-0.5: [B, 128, 4, 49]
    xp = x.reshape(B, POOL, 8, POOL, 8, C).mean(axis=(2, 4)).reshape(B, AGENT, C)
    xp = xp * 0.125
    xpT = np.ascontiguousarray(
        xp.reshape(B, AGENT, 4, 128).transpose(0, 3, 2, 1)).astype(NPBF)

    q_w = np.asarray(inputs['q_w'], np.float32)
    kv_w = np.asarray(inputs['kv_w'], np.float32)
    proj_w = np.asarray(inputs['proj_w'], np.float32)
    proj_b = np.asarray(inputs['proj_b'], np.float32)
    dwc_w = np.asarray(inputs['dwc_w'], np.float32)
    dwc_b = np.asarray(inputs['dwc_b'], np.float32)

    pb1 = _bilinear_resize_np(np.asarray(inputs['an_bias'], np.float32), H_, W_)
    pb1 = pb1.reshape(HEADS, AGENT, N_)
    pb1 = pb1 + (np.asarray(inputs['ah_bias'], np.float32)
                 + np.asarray(inputs['aw_bias'], np.float32)).reshape(HEADS, AGENT, N_)
    ab1 = _bilinear_resize_np(np.asarray(inputs['na_bias'], np.float32), H_, W_)
    ab1 = ab1.reshape(HEADS, AGENT, N_).transpose(0, 2, 1)
    pb2 = ab1 + (np.asarray(inputs['ha_bias'], np.float32)
                 + np.asarray(inputs['wa_bias'], np.float32)).reshape(HEADS, N_, AGENT)

    # eb1: [25, 128, 512] = exp(pb1) in N-layout tiles, col = 128*pr + 64*j + a
    e1p = np.zeros((N_, 4, 2, 64), np.float32)
    e1p[:, :, :, :AGENT] = np.exp(pb1).reshape(4, 2, AGENT, N_).transpose(3, 0, 1, 2)
    e1p = e1p.reshape(N_, 512)
    e1p = np.concatenate([e1p, np.zeros((3200 - N_, 512), np.float32)], 0)
    eb1 = e1p.reshape(25, 128, 512).astype(NPBF)
    # eb2: [4, 7, 128, 448] = exp(pb2) in T-layout tiles, row = 64*j + a
    e2p = np.zeros((4, 2, 64, N_), np.float32)
    e2p[:, :, :AGENT, :] = np.exp(pb2).reshape(4, 2, N_, AGENT).transpose(0, 1, 3, 2)
    eb2 = e2p.reshape(4, 128, 7, 448).transpose(0, 2, 1, 3).copy().astype(NPBF)

    dwcd = np.zeros((128, 36, 128), np.float32)
    p = np.arange(128)
    for ct in range(4):
        for ti, (dy, dx) in enumerate([(dy, dx) for dy in (-1, 0, 1)
                                       for dx in (-1, 0, 1)]):
            dwcd[p, ct * 9 + ti, p] = dwc_w[128 * ct + p, 0, dy + 1, dx + 1]

    # stage-2 denominator matrices: obd8 [128, 4, 8], obc8 [4, 8, 128]
    obd8 = np.zeros((128, 4, 8), np.float32)
    obc8 = np.zeros((8, 4, 128), np.float32)
    for pr in range(4):
        for j in range(2):
            obd8[64 * j:64 * j + AGENT, pr, 2 * pr + j] = 1.0
            obc8[2 * pr + j, pr, 64 * j:64 * j + 64] = 1.0

    common = dict(
        wq=_pack_w(q_w), wk=_pack_w(kv_w[:C]), wv=_pack_w(kv_w[C:]),
        wp=_pack_w(proj_w),
        eb1=eb1, eb2=eb2,
        dwcd=dwcd.astype(NPBF),
        dwcb=np.ascontiguousarray(dwc_b.reshape(4, 128).T).astype(np.float32),
        pjb=proj_b.reshape(1, C).astype(NPBF),
        ones_col=np.ones((128, 1), NPBF),
        ones_row=np.ones((1, 128), NPBF),
        obd8=obd8.astype(NPBF), obc8=obc8.astype(NPBF),
        padind=_padind(),
        zrow=np.zeros((1, C), NPBF),
    )
    in_maps = [dict(common,
                    xT=np.ascontiguousarray(xT[2 * i:2 * i + 2]),
                    xpT=np.ascontiguousarray(xpT[2 * i:2 * i + 2]))
               for i in range(NCORES)]
    return in_maps


import concourse.bass as bass
from concourse import bacc
import concourse.mybir as mybir
import concourse.tile as tile

BF = mybir.dt.bfloat16
F32 = mybir.dt.float32
AF = mybir.ActivationFunctionType
OP = mybir.AluOpType

C = 512
N = 3136            # 56*56 tokens
P = 128
NPAIR = 4           # head pairs; pair p = heads 2p, 2p+1 = channel tile p
NT = 7              # 448-token n-tiles
TT = 25             # 128-token tok-tiles (last has 64 rows)
CHUNK_OFF = [0, 896, 1792, 2688]
CHUNK_LEN = [896, 896, 896, 448]


def tok_rows(tt):
    return 64 if tt == TT - 1 else 128


def build_kernel(n_img=2, reps=1):
    nc = bacc.Bacc("TRN2", target_bir_lowering=False, debug=False, num_devices=8)

    xT = nc.dram_tensor("xT", [n_img, P, 4, N], BF, kind="ExternalInput")
    xpT = nc.dram_tensor("xpT", [n_img, P, 4, AGENT], BF, kind="ExternalInput")
    wq = nc.dram_tensor("wq", [P, 4, C], BF, kind="ExternalInput")
    wk = nc.dram_tensor("wk", [P, 4, C], BF, kind="ExternalInput")
    wv = nc.dram_tensor("wv", [P, 4, C], BF, kind="ExternalInput")
    wp = nc.dram_tensor("wp", [P, 4, C], BF, kind="ExternalInput")
    eb1 = nc.dram_tensor("eb1", [TT, P, 512], BF, kind="ExternalInput")
    eb1r = eb1.rearrange("t p f -> p t f")
    eb2 = nc.dram_tensor("eb2", [NPAIR, NT, P, 448], BF, kind="ExternalInput")
    eb2r = eb2.rearrange("r t p f -> p r t f")
    dwcd = nc.dram_tensor("dwcd", [P, 36, P], BF, kind="ExternalInput")
    dwcb = nc.dram_tensor("dwcb", [P, 4], F32, kind="ExternalInput")
    pjb = nc.dram_tensor("pjb", [1, C], BF, kind="ExternalInput")
    ones_col = nc.dram_tensor("ones_col", [P, 1], BF, kind="ExternalInput")
    ones_row = nc.dram_tensor("ones_row", [1, P], BF, kind="ExternalInput")
    obd8 = nc.dram_tensor("obd8", [P, 4, 8], BF, kind="ExternalInput")
    obc8 = nc.dram_tensor("obc8", [8, NPAIR, P], BF, kind="ExternalInput")
    padind = nc.dram_tensor("padind", [P, 4], F32, kind="ExternalInput")
    zrow = nc.dram_tensor("zrow", [1, C], BF, kind="ExternalInput")

    out = nc.dram_tensor("out", [n_img, N, C], BF, kind="ExternalOutput")

    with tile.TileContext(nc) as tc:
        with (
            tc.tile_pool(name="const", bufs=1) as cpool,
            tc.tile_pool(name="big", bufs=1) as bigp,
            tc.tile_pool(name="big4", bufs=3) as big4p,
            tc.tile_pool(name="ebs", bufs=4) as ebp,
            tc.tile_pool(name="eb2s", bufs=2) as ebp2,
            tc.tile_pool(name="sm", bufs=3) as smp,
            tc.tile_pool(name="e2p", bufs=6) as e2pool,
            tc.tile_pool(name="s1", bufs=1) as s1p,
            tc.tile_pool(name="outp", bufs=2) as outp,
            tc.tile_pool(name="drm", bufs=1, space="DRAM") as drm,
            tc.tile_pool(name="psA", bufs=4, space="PSUM") as psA,      # [128,512]
            tc.tile_pool(name="psS1", bufs=2, space="PSUM") as psS1,    # stage1/denoms/bc
            tc.tile_pool(name="psAcc", bufs=1, space="PSUM") as psAcc,  # av + d1
        ):
            # ---- constants (loaded once; k/v/q weights first) ----
            wq_s = cpool.tile([P, 4, C], BF, tag="wq")
            wk_s = cpool.tile([P, 4, C], BF, tag="wk")
            wv_s = cpool.tile([P, 4, C], BF, tag="wv")
            wp_s = cpool.tile([P, 4, C], BF, tag="wp")
            nc.sync.dma_start(wk_s[:], wk[:])
            nc.sync.dma_start(wv_s[:], wv[:])
            nc.sync.dma_start(wq_s[:], wq[:])
            nc.sync.dma_start(wp_s[:], wp[:])
            dwcd_s = cpool.tile([P, 36, P], BF, tag="dwcd")
            nc.sync.dma_start(dwcd_s[:], dwcd[:])
            dwcb_s = cpool.tile([P, 4], F32, tag="dwcb")
            nc.sync.dma_start(dwcb_s[:], dwcb[:])
            pjb_s = cpool.tile([1, C], BF, tag="pjb")
            nc.sync.dma_start(pjb_s[:], pjb[:])
            oc_s = cpool.tile([P, 1], BF, tag="ones_col")
            nc.sync.dma_start(oc_s[:], ones_col[:])
            or_s = cpool.tile([1, P], BF, tag="ones_row")
            nc.sync.dma_start(or_s[:], ones_row[:])
            obd_s = cpool.tile([P, 4, 8], BF, tag="obd8")
            nc.sync.dma_start(obd_s[:], obd8[:])
            obc_s = cpool.tile([8, NPAIR, P], BF, tag="obc8")
            nc.sync.dma_start(obc_s[:], obc8[:])
            pad_s = cpool.tile([P, 4], F32, tag="padind")
            nc.sync.dma_start(pad_s[:], padind[:])
            zr_s = cpool.tile([1, C], BF, tag="zrow")
            nc.sync.dma_start(zr_s[:], zrow[:])

            # padded v image, zeroed once; interior rewritten per image
            vtp = bigp.tile([P, 4, 58 * 58], BF, tag="vtp")
            nc.gpsimd.memset(vtp[:], 0.0)
            vtp3 = vtp.rearrange("p c (r q) -> p c r q", q=58)

            evac_i = [0]   # evacuation engine alternator

            def evac(dst, src):
                if evac_i[0] % 3 == 0:
                    nc.vector.tensor_copy(dst, src)
                else:
                    nc.scalar.copy(dst, src)
                evac_i[0] += 1

            for img in [i for _ in range(reps) for i in range(n_img)]:
                # ---- per-image big SBUF tensors ----
                xT_s = big4p.tile([P, 4, N], BF, tag="big4")
                for nsc in range(4):
                    coff, clen = CHUNK_OFF[nsc], CHUNK_LEN[nsc]
                    nc.sync.dma_start(xT_s[:, :, coff:coff + clen],
                                      xT[img, :, :, coff:coff + clen])
                kT = big4p.tile([P, 4, N], BF, tag="big4")
                qT = big4p.tile([P, 4, N], BF, tag="big4")
                vN = bigp.tile([P, TT, C], BF, tag="vN")

                # ---- agent tokens -> block-diagonal bd (pre-scaled on host) ----
                xp_s = s1p.tile([P, 4, AGENT], BF, tag="xp")
                nc.sync.dma_start(xp_s[:], xpT[img])
                ps_ah = psS1.tile([P, 512], F32, tag="psS1")
                for pr in range(NPAIR):
                    for kt in range(4):
                        nc.tensor.matmul(
                            ps_ah[:, 128 * pr:128 * pr + AGENT],
                            lhsT=wq_s[:, kt, 128 * pr:128 * pr + 128],
                            rhs=xp_s[:, kt, :],
                            start=(pr == 0 and kt == 0),
                            stop=(pr == 3 and kt == 3),
                            skip_group_check=True)
                bd = []
                for pr in range(NPAIR):
                    b = s1p.tile([P, P], BF, tag=f"bd{pr}")
                    nc.gpsimd.memset(b[:], 0.0)
                    nc.vector.tensor_copy(b[0:64, 0:AGENT],
                                          ps_ah[0:64, 128 * pr:128 * pr + AGENT])
                    nc.vector.tensor_copy(b[64:128, 64:64 + AGENT],
                                          ps_ah[64:128, 128 * pr:128 * pr + AGENT])
                    bd.append(b)

                # ================= QKV GEMMs =================
                # per chunk: k, v(T, padded), vN, q — so stage 1 can start early
                for nsc in range(4):
                    clen = CHUNK_LEN[nsc]
                    coff = CHUNK_OFF[nsc]
                    nsub = clen // 448
                    for m in range(12):
                        # order: k (0-3), v (4-7), q (8-11)
                        w_s, dst = (wk_s, kT) if m < 4 else (
                            (wv_s, None) if m < 8 else (wq_s, qT))
                        mt = m % 4
                        for sub in range(nsub):
                            ps = psA.tile([P, 512], F32, tag="psA")
                            for kt in range(4):
                                nc.tensor.matmul(
                                    ps[:, :448],
                                    lhsT=w_s[:, kt, 128 * mt:128 * mt + 128],
                                    rhs=xT_s[:, kt, coff + 448 * sub:coff + 448 * sub + 448],
                                    start=(kt == 0), stop=(kt == 3))
                            nt = (coff + 448 * sub) // 448
                            if dst is not None:
                                evac(dst[:, mt, 448 * nt:448 * nt + 448], ps[:, :448])
                            else:
                                evac(vtp3[:, mt, 8 * nt + 1:8 * nt + 9, 1:57],
                                     ps[:, :448])
                        if m == 7:
                            # vN for this chunk's tok-tiles (after v, before q)
                            for tl in range(clen // 128 + (1 if nsc == 3 else 0)):
                                tt = (coff // 128) + tl
                                r = tok_rows(tt)
                                ps = psA.tile([P, 512], F32, tag="psA")
                                for kt in range(4):
                                    nc.tensor.matmul(
                                        ps[:r, :],
                                        lhsT=xT_s[:, kt, coff + 128 * tl:coff + 128 * tl + r],
                                        rhs=wv_s[:, kt, :],
                                        start=(kt == 0), stop=(kt == 3))
                                evac(vN[:r, tt, :], ps[:r, :])

                # ================= stage 1 (N-layout) =================
                ps_d1 = psAcc.tile([1, 512], F32, tag="d1")
                ps_av = psAcc.tile([P, 512], F32, tag="av")
                # zero-init av bank so later matmuls can pure-accumulate
                nc.tensor.matmul(ps_av[:], lhsT=zr_s[:, :P], rhs=zr_s[:],
                                 start=True, stop=False, skip_group_check=True)
                for tt in range(TT):
                    r = tok_rows(tt)
                    ebt = ebp.tile([P, 512], BF, tag="eb1")
                    nc.sync.dma_start(ebt[:r, :], eb1[tt, :r, :])
                    ps1 = psS1.tile([P, 512], F32, tag="psS1")
                    for pr in range(NPAIR):
                        nc.tensor.matmul(
                            ps1[:r, 128 * pr:128 * pr + 128],
                            lhsT=kT[:, pr, 128 * tt:128 * tt + r],
                            rhs=bd[pr][:],
                            start=(pr == 0), stop=(pr == 3),
                            skip_group_check=True)
                    er = smp.tile([P, 512], BF, tag="er1")
                    nc.scalar.activation(er[:r, :], ps1[:r, :], AF.Exp)
                    e1 = smp.tile([P, 512], BF, tag="e1")
                    nc.vector.tensor_tensor(e1[:r, :], er[:r, :],
                                            ebt[:r, :], OP.mult)
                    nc.tensor.matmul(ps_d1[:], lhsT=oc_s[:r, :], rhs=e1[:r, :],
                                     start=(tt == 0), stop=(tt == TT - 1),
                                     skip_group_check=True)
                    for pr in range(NPAIR):
                        nc.tensor.matmul(
                            ps_av[:, 128 * pr:128 * pr + 128],
                            lhsT=e1[:r, 128 * pr:128 * pr + 128],
                            rhs=vN[:r, tt, 128 * pr:128 * pr + 128],
                            start=False, stop=(tt == TT - 1 and pr == 3),
                            skip_group_check=True)

                # denominators -> [98,4] via small sbuf->sbuf DMA transpose
                dn = s1p.tile([1, 512], F32, tag="dn")
                nc.scalar.copy(dn[:], ps_d1[:])
                dn_d = drm.tile([1, 512], F32, tag="dn_d")
                nc.sync.dma_start(dn_d[:], dn[:])
                dnT = s1p.tile([P, 4], F32, tag="dnT")
                nc.sync.dma_start(
                    dnT[:], dn_d[0, :].rearrange("(f p) -> p f", p=P))
                dnT2 = s1p.tile([P, 4], F32, tag="dnT2")
                nc.vector.tensor_tensor(dnT2[:], dnT[:], pad_s[:], OP.add)
                r1 = s1p.tile([P, 4], F32, tag="r1")
                nc.vector.reciprocal(out=r1[:], in_=dnT2[:])
                bdav = []
                for pr in range(NPAIR):
                    b = s1p.tile([P, P], BF, tag=f"bdav{pr}")
                    nc.gpsimd.memset(b[:], 0.0)
                    nc.vector.tensor_scalar(
                        b[0:64, 0:64], ps_av[0:64, 128 * pr:128 * pr + 64],
                        r1[0:64, pr:pr + 1], None, OP.mult)
                    nc.vector.tensor_scalar(
                        b[64:128, 64:128], ps_av[64:128, 128 * pr + 64:128 * pr + 128],
                        r1[64:128, pr:pr + 1], None, OP.mult)
                    bdav.append(b)

                # ================= stage 2 (T-layout) + dwc =================
                zT = big4p.tile([P, 4, N], BF, tag="big4")

                ot = [None]

                def proj_tile(tt):
                    r = tok_rows(tt)
                    pso = psA.tile([P, 512], F32, tag="psA")
                    for ct in range(4):
                        nc.tensor.matmul(
                            pso[:r, :],
                            lhsT=zT[:, ct, 128 * tt:128 * tt + r],
                            rhs=wp_s[:, ct, :],
                            start=(ct == 0), stop=(ct == 3))
                    if tt % 4 == 0:
                        otn = outp.tile([P, 4, C], BF, tag="ot", name=f"ot{tt}")
                        ot[0] = otn
                    evac(ot[0][:r, tt % 4, :], pso[:r, :])
                    if tt % 4 == 3 or tt == TT - 1:
                        g0 = (tt // 4) * 4
                        ntl = tt - g0 + 1
                        if tt == TT - 1:
                            nc.sync.dma_start(
                                out[img, 128 * tt:128 * tt + 64, :],
                                ot[0][:64, 0, :])
                        else:
                            nc.sync.dma_start(
                                out[img, 128 * g0:128 * (g0 + ntl), :].rearrange(
                                    "(t p) c -> p t c", p=P),
                                ot[0][:, :ntl, :])

                proj_done = 0
                for nt in range(NT):
                    ns = slice(448 * nt, 448 * nt + 448)
                    ebt2 = ebp2.tile([P, 4, 448], BF, tag="eb2")
                    nc.sync.dma_start(ebt2[:], eb2r[:, :, nt, :])
                    psd = psS1.tile([8, 448], F32, tag="psS1")
                    e2s = []
                    for pr in range(NPAIR):
                        ps2 = psA.tile([P, 512], F32, tag="psA")
                        nc.tensor.matmul(ps2[:, :448], lhsT=bd[pr][:],
                                         rhs=qT[:, pr, ns], start=True, stop=True)
                        er2 = smp.tile([P, 448], BF, tag="er2")
                        nc.scalar.activation(er2[:], ps2[:, :448], AF.Exp)
                        e2 = e2pool.tile([P, 448], BF, tag="e2")
                        nc.vector.tensor_tensor(e2[:], er2[:], ebt2[:, pr, :], OP.mult)
                        nc.tensor.matmul(psd[:], lhsT=obd_s[:, pr, :], rhs=e2[:],
                                         start=(pr == 0), stop=(pr == 3),
                                         skip_group_check=True)
                        e2s.append(e2)
                    r2b = smp.tile([8, 448], BF, tag="r2b")
                    with nc.allow_low_precision(
                            reason="softmax denom reciprocal fits bf16"):
                        nc.vector.reciprocal(out=r2b[:], in_=psd[:])
                    for pr in range(NPAIR):
                        ps_bc = psS1.tile([P, 512], F32, tag="psS1")
                        nc.tensor.matmul(ps_bc[:, :448], lhsT=obc_s[:, pr, :],
                                         rhs=r2b[:], start=True, stop=True)
                        e2n = smp.tile([P, 448], BF, tag="e2n")
                        nc.vector.tensor_tensor(e2n[:], e2s[pr][:], ps_bc[:, :448],
                                                OP.mult)
                        # out accumulation: 9 dwc taps (independent of stage 1)
                        # first, then attention last
                        psz = psA.tile([P, 512], F32, tag="psA")
                        ti = 0
                        for dy in (-1, 0, 1):
                            for dx in (-1, 0, 1):
                                nc.tensor.matmul(
                                    psz[:, :448],
                                    lhsT=dwcd_s[:, pr * 9 + ti, :],
                                    rhs=vtp3[:, pr, 8 * nt + 1 + dy:8 * nt + 9 + dy,
                                             1 + dx:57 + dx],
                                    start=(ti == 0), stop=False,
                                    skip_group_check=True)
                                ti += 1
                        nc.tensor.matmul(psz[:, :448], lhsT=bdav[pr][:], rhs=e2n[:],
                                         start=False, stop=True,
                                         skip_group_check=True)
                        nc.scalar.activation(zT[:, pr, ns], psz[:, :448],
                                             AF.Identity, bias=dwcb_s[:, pr:pr + 1])
                    # proj for token tiles fully covered by stage-2 so far
                    ready = (448 * (nt + 1)) // 128
                    if nt == NT - 1:
                        ready = TT
                    while proj_done < ready:
                        proj_tile(proj_done)
                        proj_done += 1

    nc.compile()
    return nc


_CACHE = {}


def _get_nc():
    if 'nc' not in _CACHE:
        _CACHE['nc'] = build_kernel(2)
    return _CACHE['nc']


def kernel(x, H, W, q_w, kv_w, proj_w, proj_b, dwc_w, dwc_b,
           an_bias, na_bias, ah_bias, aw_bias, ha_bias, wa_bias):
    from concourse.bass_utils import run_bass_kernel_spmd
    in_maps = _prepare(dict(
        x=x, q_w=q_w, kv_w=kv_w, proj_w=proj_w, proj_b=proj_b,
        dwc_w=dwc_w, dwc_b=dwc_b, an_bias=an_bias, na_bias=na_bias,
        ah_bias=ah_bias, aw_bias=aw_bias, ha_bias=ha_bias, wa_bias=wa_bias))
    nc = _get_nc()
    res = run_bass_kernel_spmd(nc, in_maps, core_ids=list(range(NCORES)))
    out = np.stack([np.asarray(res.results[i]['out'], np.float32)
                    for i in range(NCORES)])
    out = out.reshape(B, N_, C) + np.asarray(proj_b, np.float32)[None, None, :]
    return out.reshape(B, C, H_, W_)
